# revision 16
# baseline (speedup 1.0000x reference)
"""Trainium2 Bass kernel for nn_DGLossVer1 (SO(3) gyro loss).

Math: the product of 16 (or 32) small-rotation exponentials exp(dt*w_i) is
composed via the 2nd-order BCH formula Z = dt*S + (dt^2/2)*C with
S = sum(u_i), C = sum_{i<j} u_i x u_j, computed by a pairwise tree
(C_AB = C_A + C_B + S_A x S_B).  The block rotation is kept as an
UNNORMALIZED quaternion (1, tan(|Z|/2)/|Z| * Z); everything downstream
(relative rotation, log) is scale-invariant, so no normalization anywhere.
The log mirrors the reference: ang = arccos(clip(2w^2-1)), coef =
0.5*ang/sin(ang), phi = coef*4*w*v (all in the scale-free form), with
arccos(c) = pi/2 - arctan(c*rsqrt(1-c^2)).  rsqrt = bit-trick seed + 2
Newton iterations (ACT Rsqrt/Reciprocal are banned for accuracy).

Sharding: pure data-parallel, 8 sequences per core; each core returns two
partial Huber sums per partition; host does the tiny weighted reduction.
"""
import numpy as np

P = 128
DT = 0.005
WLOSS = 1.0e6
HUBER = 0.005
N0 = 5
NSEQ, T = 64, 32768
NCORES = 8
SPC = NSEQ // NCORES          # sequences per core
STEPS = SPC * T // P          # 2048 steps per partition
WCOLS = STEPS * 3             # 6144
NB16 = STEPS // 16            # 128 16-blocks per partition
NB32 = STEPS // 32            # 64
DCOLS = NB16 * 3              # 384
N1 = STEPS // 2               # 1024 pairs per partition

_CACHE = {}


def _build(debug=False):
    import concourse.bass as bass
    import concourse.tile as tile
    import concourse.mybir as mybir
    from concourse import bacc

    f32 = mybir.dt.float32
    i32 = mybir.dt.int32
    AF = mybir.ActivationFunctionType
    OP = mybir.AluOpType
    AX = mybir.AxisListType

    nc = bacc.Bacc(None)
    w_d = nc.declare_dram_parameter("w", [P, WCOLS], f32, isOutput=False)
    d_d = nc.declare_dram_parameter("d", [P, DCOLS], f32, isOutput=False)
    o_d = nc.declare_dram_parameter("out", [P, 2], f32, isOutput=True)

    TT, TS, STT = OP.mult, None, None  # readability only

    with tile.TileContext(nc) as tc:
        with tc.tile_pool(name="main", bufs=1) as pool:
            # ---- input DMA (w in 2 chunks for overlap) ----
            w0 = pool.tile([P, WCOLS // 2], f32)
            w1 = pool.tile([P, WCOLS // 2], f32)
            d = pool.tile([P, DCOLS], f32)
            nc.sync.dma_start(w0[:], w_d[:, 0:WCOLS // 2])
            nc.sync.dma_start(d[:], d_d[:])
            nc.sync.dma_start(w1[:], w_d[:, WCOLS // 2:])

            hpi = pool.tile([P, 1], f32)
            nc.gpsimd.memset(hpi[:], float(np.pi / 2))
            fpi = pool.tile([P, 1], f32)
            nc.gpsimd.memset(fpi[:], float(np.pi))

            def rsqrt2(x_ap, n, out_t, scr_t, eng_tt=nc.vector, iters=2):
                """out = rsqrt(x) via bit trick + Newton iters.
                x_ap: (P, n) AP (positive); out_t/scr_t: (P, n) f32 tiles."""
                oi = out_t[:, 0:n].bitcast(i32)
                si = scr_t[:, 0:n].bitcast(i32)
                nc.vector.tensor_scalar(si, x_ap.bitcast(i32), 1, -1,
                                        OP.arith_shift_right, OP.bitwise_xor)
                nc.vector.tensor_scalar(oi, si, 0x5F3759E0, None, OP.add)
                y = out_t[:, 0:n]
                s = scr_t[:, 0:n]
                for _ in range(iters):
                    eng_tt.tensor_tensor(s, y, y, OP.mult)
                    eng_tt.tensor_tensor(s, s, x_ap, OP.mult)
                    nc.gpsimd.tensor_scalar(s, s, -0.5, 1.5, OP.mult, OP.add)
                    eng_tt.tensor_tensor(y, y, s, OP.mult)
                return y

            # ---- k1: pairs of steps from AoS w chunks ----
            # S1/C1: (P, 3*N1) comp-major [x | y | z]
            S1 = pool.tile([P, 3 * N1], f32)
            C1 = pool.tile([P, 3 * N1], f32)
            CR = pool.tile([P, 3 * N1], f32)  # cross scratch, reused at all levels
            H = N1 // 2  # pairs per chunk
            for h, wt in ((0, w0), (1, w1)):
                def we(c):  # even-step comp c view, this chunk
                    return wt[:, c::6]
                def wo(c):
                    return wt[:, c + 3::6]
                for c in range(3):
                    eng = nc.gpsimd if c == 0 else nc.vector
                    a, b = (c + 1) % 3, (c + 2) % 3
                    dst = C1[:, c * N1 + h * H:c * N1 + (h + 1) * H]
                    scr = CR[:, c * N1 + h * H:c * N1 + (h + 1) * H]
                    eng.tensor_tensor(dst, we(a), wo(b), OP.mult)
                    eng.tensor_tensor(scr, we(b), wo(a), OP.mult)
                    eng.tensor_tensor(dst, dst, scr, OP.subtract)
                # S1 3-comp merged: out [(N1,3),(1,H)] ; in [(1,3),(6,H)]
                s_out = S1.rearrange("p (c n) -> p c n", c=3)[:, :, h * H:(h + 1) * H]
                w3 = wt.rearrange("p (t c) -> p c t", c=3)  # (P,3,2H*?) steps
                # even steps: t stride 2 -> use explicit AP via step slicing
                nc.vector.tensor_tensor(
                    s_out, w3[:, :, 0:2 * H:2], w3[:, :, 1:2 * H:2], OP.add)

            # ---- k2..k5 ----
            def level(Sp, Cp, n_in, Sn, Cn, s_off=0, d_off=0):
                """pairwise combine: n_in -> n_in//2 per comp.
                Sp/Cp views are comp-blocked tiles with block stride bs_in,
                writing into Sn/Cn at comp block stride bs_out + d_off."""
                n = n_in // 2
                for c in range(3):
                    eng = nc.gpsimd if c == 0 else nc.vector
                    a, b = (c + 1) % 3, (c + 2) % 3
                    def ev(t, cc, bs):
                        return t.rearrange("p (c n) -> p c n", c=3)[:, cc, s_off:s_off + n_in:2]
                    def od(t, cc, bs):
                        return t.rearrange("p (c n) -> p c n", c=3)[:, cc, s_off + 1:s_off + n_in:2]
                    dst = Cn.rearrange("p (c n) -> p c n", c=3)[:, c, d_off:d_off + n]
                    scr = CR[:, c * N1:c * N1 + n]
                    eng.tensor_tensor(dst, ev(Sp, a, 0), od(Sp, b, 0), OP.mult)
                    eng.tensor_tensor(scr, ev(Sp, b, 0), od(Sp, a, 0), OP.mult)
                    eng.tensor_tensor(dst, dst, scr, OP.subtract)
                # C += Ce + Co (3-comp merged)
                c3 = Cn.rearrange("p (c n) -> p c n", c=3)[:, :, d_off:d_off + n]
                cp3 = Cp.rearrange("p (c n) -> p c n", c=3)
                s3 = Sn.rearrange("p (c n) -> p c n", c=3)[:, :, d_off:d_off + n]
                sp3 = Sp.rearrange("p (c n) -> p c n", c=3)
                ce = cp3[:, :, s_off:s_off + n_in:2]
                co = cp3[:, :, s_off + 1:s_off + n_in:2]
                sc = CR.rearrange("p (c n) -> p c n", c=3)[:, :, 0:n]
                nc.vector.tensor_tensor(sc, ce, co, OP.add)
                nc.vector.tensor_tensor(c3, c3, sc, OP.add)
                nc.vector.tensor_tensor(
                    s3, sp3[:, :, s_off:s_off + n_in:2],
                    sp3[:, :, s_off + 1:s_off + n_in:2], OP.add)

            S2 = pool.tile([P, 3 * 512], f32)
            C2 = pool.tile([P, 3 * 512], f32)
            S3 = pool.tile([P, 3 * 256], f32)
            C3 = pool.tile([P, 3 * 256], f32)
            # S45/C45: (P, 3*192): per comp [16-level 0:128 | 32-level 128:192]
            S45 = pool.tile([P, 3 * 192], f32)
            C45 = pool.tile([P, 3 * 192], f32)
            level(S1, C1, N1, S2, C2)
            level(S2, C2, 512, S3, C3)
            level(S3, C3, 256, S45, C45)          # k4 -> NB16 at d_off 0
            level(S45, C45, 128, S45, C45, d_off=128)  # k5 reads [0:128) writes [128:192)

            # ---- Z, tan-poly, gh (homogeneous quat vector part) ----
            Z = pool.tile([P, 3 * 192], f32)
            nc.vector.scalar_tensor_tensor(Z[:], C45[:], DT / 2, S45[:],
                                           OP.mult, OP.add)
            sqz = pool.tile([P, 3 * 192], f32)
            nc.scalar.activation(sqz[:], Z[:], AF.Square)
            n2z = pool.tile([P, 192], f32)
            nc.vector.tensor_tensor(n2z[:], sqz[:, 0:192], sqz[:, 192:384], OP.add)
            nc.vector.tensor_tensor(n2z[:], n2z[:], sqz[:, 384:576], OP.add)
            tp = pool.tile([P, 192], f32)
            nc.gpsimd.tensor_scalar(tp[:], n2z[:], DT ** 4 / 240, DT ** 2 / 24,
                                    OP.mult, OP.add)
            nc.vector.tensor_tensor(tp[:], tp[:], n2z[:], OP.mult)
            nc.gpsimd.tensor_scalar(tp[:], tp[:], 0.5, DT, OP.add, OP.mult)
            gh = pool.tile([P, 3 * 192], f32)
            tpb = tp[:].unsqueeze(1).broadcast_to([P, 3, 192])
            nc.vector.tensor_tensor(gh.rearrange("p (c n) -> p c n", c=3),
                                    tpb, Z.rearrange("p (c n) -> p c n", c=3),
                                    OP.mult)

            # ---- d16 exp (true unit quats via Sin table) ----
            # dq: (P, 4*192): [w | x | y | z], each [d16 0:128 | d32 128:192]
            dq = pool.tile([P, 4 * 192], f32)
            sqd = pool.tile([P, DCOLS], f32)
            nc.scalar.activation(sqd[:], d[:], AF.Square)
            n2d = pool.tile([P, NB16], f32)
            nc.vector.tensor_tensor(n2d[:], sqd[:, 0:DCOLS:3], sqd[:, 1:DCOLS:3], OP.add)
            nc.vector.tensor_tensor(n2d[:], n2d[:], sqd[:, 2:DCOLS:3], OP.add)
            nc.gpsimd.tensor_scalar(n2d[:], n2d[:], 1e-30, None, OP.max)
            y1t = pool.tile([P, NB16], f32)
            scr1 = pool.tile([P, NB16], f32)
            y1 = rsqrt2(n2d[:], NB16, y1t, scr1, iters=3)
            th = pool.tile([P, NB16], f32)
            nc.vector.tensor_tensor(th[:], n2d[:], y1, OP.mult)
            # cos(th/2) = sin(pi/2 - th/2); sin(th/2) = sin(pi - th/2)
            # (keeps the argument inside the Sin table's [-pi, pi] domain)
            nc.scalar.activation(dq[:, 0:128], th[:], AF.Sin, bias=hpi[:], scale=-0.5)
            s0 = pool.tile([P, NB16], f32)
            nc.scalar.activation(s0[:], th[:], AF.Sin, bias=fpi[:], scale=-0.5)
            nc.vector.tensor_tensor(s0[:], s0[:], y1, OP.mult)
            s0b = s0[:].unsqueeze(1).broadcast_to([P, 3, NB16])
            dqv16 = dq.rearrange("p (c n) -> p c n", c=4)[:, 1:4, 0:128]
            d3 = d.rearrange("p (j c) -> p c j", c=3)
            nc.vector.tensor_tensor(dqv16, s0b, d3, OP.mult)

            # ---- d32 = qmul(d16 even, d16 odd) into dq[:, *, 128:192] ----
            dq4 = dq.rearrange("p (c n) -> p c n", c=4)
            q1 = dq4[:, :, 0:128:2]
            q2 = dq4[:, :, 1:128:2]
            pp = pool.tile([P, 4 * NB32], f32)
            pp4 = pp.rearrange("p (c n) -> p c n", c=4)
            nc.vector.tensor_tensor(pp4, q1, q2, OP.mult)
            w32 = dq[:, 128:192]
            nc.vector.tensor_tensor(w32, pp[:, 0:64], pp[:, 64:128], OP.subtract)
            nc.vector.tensor_tensor(w32, w32, pp[:, 128:192], OP.subtract)
            nc.vector.tensor_tensor(w32, w32, pp[:, 192:256], OP.subtract)
            w1b = dq[:, 0:128:2].unsqueeze(1).broadcast_to([P, 3, NB32])
            w2b = dq[:, 1:128:2].unsqueeze(1).broadcast_to([P, 3, NB32])
            v1 = dq4[:, 1:4, 0:128:2]
            v2 = dq4[:, 1:4, 1:128:2]
            t1 = pool.tile([P, 3 * NB32], f32)
            t13 = t1.rearrange("p (c n) -> p c n", c=3)
            t2 = pool.tile([P, 3 * NB32], f32)
            t23 = t2.rearrange("p (c n) -> p c n", c=3)
            nc.vector.tensor_tensor(t13, w1b, v2, OP.mult)
            nc.vector.tensor_tensor(t23, w2b, v1, OP.mult)
            nc.vector.tensor_tensor(t1[:], t1[:], t2[:], OP.add)
            cr32 = pool.tile([P, 3 * NB32], f32)
            for c in range(3):
                a, b = (c + 1) % 3, (c + 2) % 3
                dst = cr32[:, c * NB32:(c + 1) * NB32]
                scr = t2[:, c * NB32:(c + 1) * NB32]
                nc.gpsimd.tensor_tensor(dst, v1[:, a], v2[:, b], OP.mult)
                nc.gpsimd.tensor_tensor(scr, v1[:, b], v2[:, a], OP.mult)
                nc.gpsimd.tensor_tensor(dst, dst, scr, OP.subtract)
            nc.vector.tensor_tensor(t1[:], t1[:], cr32[:], OP.add)
            dqv32 = dq4[:, 1:4, 128:192]
            nc.vector.tensor_copy(dqv32, t13)

            # ---- rel = conj(1, gh) x dq   (width 192) ----
            gh3 = gh.rearrange("p (c n) -> p c n", c=3)
            dqv = dq4[:, 1:4, :]
            dm = pool.tile([P, 3 * 192], f32)
            nc.vector.tensor_tensor(dm.rearrange("p (c n) -> p c n", c=3),
                                    gh3, dqv, OP.mult)
            rw = pool.tile([P, 192], f32)
            nc.vector.tensor_tensor(rw[:], dm[:, 0:192], dm[:, 192:384], OP.add)
            nc.vector.tensor_tensor(rw[:], rw[:], dm[:, 384:576], OP.add)
            nc.vector.tensor_tensor(rw[:], rw[:], dq[:, 0:192], OP.add)
            cwb = dq[:, 0:192].unsqueeze(1).broadcast_to([P, 3, 192])
            rv = pool.tile([P, 3 * 192], f32)
            rv3 = rv.rearrange("p (c n) -> p c n", c=3)
            nc.vector.tensor_tensor(rv3, cwb, gh3, OP.mult)
            nc.vector.tensor_tensor(rv[:], dq[:, 192:], rv[:], OP.subtract)
            crr = pool.tile([P, 3 * 192], f32)
            for c in range(3):
                a, b = (c + 1) % 3, (c + 2) % 3
                dst = crr[:, c * 192:(c + 1) * 192]
                scr = dm[:, c * 192:(c + 1) * 192]
                nc.gpsimd.tensor_tensor(dst, gh3[:, a], dqv[:, b], OP.mult)
                nc.gpsimd.tensor_tensor(scr, gh3[:, b], dqv[:, a], OP.mult)
                nc.gpsimd.tensor_tensor(dst, dst, scr, OP.subtract)
            nc.vector.tensor_tensor(rv[:], rv[:], crr[:], OP.subtract)

            # ---- log (mirrors reference clip semantics, scale-free) ----
            W2 = 192
            sqv = pool.tile([P, 3 * W2], f32)
            nc.scalar.activation(sqv[:], rv[:], AF.Square)
            n2v = pool.tile([P, W2], f32)
            nc.vector.tensor_tensor(n2v[:], sqv[:, 0:192], sqv[:, 192:384], OP.add)
            nc.vector.tensor_tensor(n2v[:], n2v[:], sqv[:, 384:576], OP.add)
            w2t = pool.tile([P, W2], f32)
            nc.scalar.activation(w2t[:], rw[:], AF.Square)
            q2t = pool.tile([P, W2], f32)
            nc.vector.tensor_tensor(q2t[:], w2t[:], n2v[:], OP.add)
            rt = pool.tile([P, W2], f32)
            rscr = pool.tile([P, W2], f32)
            r = rsqrt2(q2t[:], W2, rt, rscr)
            rq = pool.tile([P, W2], f32)
            nc.vector.tensor_tensor(rq[:], r, r, OP.mult)      # ~1/q2
            # one reciprocal-Newton to bring rq to ~1-ULP: rq *= (2 - q2*rq)
            nc.vector.tensor_tensor(rscr[:], q2t[:], rq[:], OP.mult)
            nc.gpsimd.tensor_scalar(rscr[:], rscr[:], -1.0, 2.0, OP.mult, OP.add)
            nc.vector.tensor_tensor(rq[:], rq[:], rscr[:], OP.mult)
            cost = pool.tile([P, W2], f32)
            nc.vector.tensor_tensor(cost[:], w2t[:], n2v[:], OP.subtract)
            nc.vector.tensor_tensor(cost[:], cost[:], rq[:], OP.mult)
            CLIP = 1.0 - 1e-7
            nc.gpsimd.tensor_scalar(cost[:], cost[:], CLIP, -CLIP, OP.min, OP.max)
            c2t = pool.tile([P, W2], f32)
            nc.scalar.activation(c2t[:], cost[:], AF.Square)
            nc.gpsimd.tensor_scalar(c2t[:], c2t[:], -1.0, 1.0, OP.mult, OP.add)
            rs2t = pool.tile([P, W2], f32)
            rs2 = rsqrt2(c2t[:], W2, rs2t, rscr)               # 1/sin(ang)
            # F = 0.5*arccos(cos)/sin(arccos(cos)) via deg-10 poly in t=|cos|-1
            # (F is analytic at cos=1; the cos<0 half uses F(-c) and pi/2*rs2)
            KP = [0.5000000010056445, -0.1666664296147386, 0.06667585538901223,
                  -0.028433366986487976, 0.013753622162797092,
                  -0.0011196834360748097, 0.015245614903288171,
                  0.020070084287574758, 0.02282400093211004,
                  0.01299667485963209, 0.0037463467111214254]
            tpoly = pool.tile([P, W2], f32)
            nc.scalar.activation(tpoly[:], cost[:], AF.Abs)
            nc.gpsimd.tensor_scalar(tpoly[:], tpoly[:], -1.0, None, OP.add)  # t = |c|-1
            t2p = pool.tile([P, W2], f32)
            t4p = pool.tile([P, W2], f32)
            nc.vector.tensor_tensor(t2p[:], tpoly[:], tpoly[:], OP.mult)
            nc.vector.tensor_tensor(t4p[:], t2p[:], t2p[:], OP.mult)
            e0 = pool.tile([P, W2], f32)
            e1 = pool.tile([P, W2], f32)
            e2 = pool.tile([P, W2], f32)
            e3 = pool.tile([P, W2], f32)
            e4 = pool.tile([P, W2], f32)
            nc.gpsimd.tensor_scalar(e0[:], tpoly[:], KP[1], KP[0], OP.mult, OP.add)
            nc.gpsimd.tensor_scalar(e1[:], tpoly[:], KP[3], KP[2], OP.mult, OP.add)
            nc.gpsimd.tensor_scalar(e2[:], tpoly[:], KP[5], KP[4], OP.mult, OP.add)
            nc.gpsimd.tensor_scalar(e3[:], tpoly[:], KP[7], KP[6], OP.mult, OP.add)
            nc.gpsimd.tensor_scalar(e4[:], tpoly[:], KP[9], KP[8], OP.mult, OP.add)
            g2 = pool.tile([P, W2], f32)
            nc.gpsimd.tensor_scalar(g2[:], t2p[:], KP[10], None, OP.mult)
            nc.vector.tensor_tensor(g2[:], g2[:], e4[:], OP.add)     # f2
            nc.vector.tensor_tensor(e1[:], e1[:], t2p[:], OP.mult)
            nc.vector.tensor_tensor(e0[:], e0[:], e1[:], OP.add)     # f0
            nc.vector.tensor_tensor(e3[:], e3[:], t2p[:], OP.mult)
            nc.vector.tensor_tensor(e2[:], e2[:], e3[:], OP.add)     # f1
            nc.vector.tensor_tensor(g2[:], g2[:], t4p[:], OP.mult)
            nc.vector.tensor_tensor(g2[:], g2[:], e2[:], OP.add)
            nc.vector.tensor_tensor(g2[:], g2[:], t4p[:], OP.mult)
            nc.vector.tensor_tensor(g2[:], g2[:], e0[:], OP.add)     # p = F(|c|)
            sgn = pool.tile([P, W2], f32)
            nc.scalar.activation(sgn[:], cost[:], AF.Sign)
            u1 = pool.tile([P, W2], f32)
            nc.gpsimd.tensor_scalar(u1[:], sgn[:], float(-np.pi / 4),
                                    float(np.pi / 4), OP.mult, OP.add)
            nc.vector.tensor_tensor(u1[:], u1[:], rs2, OP.mult)
            nc.vector.tensor_tensor(g2[:], g2[:], sgn[:], OP.mult)
            cf = pool.tile([P, W2], f32)
            nc.vector.tensor_tensor(cf[:], u1[:], g2[:], OP.add)     # 0.5*ang/sin
            cf2 = pool.tile([P, W2], f32)
            nc.vector.scalar_tensor_tensor(cf2[:], rw[:], 4.0, rq[:], OP.mult, OP.mult)
            nc.vector.tensor_tensor(cf[:], cf[:], cf2[:], OP.mult)
            rs = pool.tile([P, 3 * W2], f32)
            cfb = cf[:].unsqueeze(1).broadcast_to([P, 3, W2])
            nc.vector.tensor_tensor(rs.rearrange("p (c n) -> p c n", c=3),
                                    cfb, rv3, OP.mult)

            # ---- huber + partial sums ----
            # The N0-skip is handled host-side: ship the skipped rs values out
            # and subtract their huber contribution from the partial sums.
            rs3 = rs.rearrange("p (c n) -> p c n", c=3)
            skip_d = nc.declare_dram_parameter("skip", [SPC, 2 * 3 * N0], f32,
                                               isOutput=True)
            nc.sync.dma_start(skip_d[:, 0:3 * N0], rs3[0:P:16, :, 0:N0])
            nc.sync.dma_start(skip_d[:, 3 * N0:], rs3[0:P:16, :, 128:128 + N0])
            xb = pool.tile([P, 3 * W2], f32)
            nc.scalar.activation(xb[:], rs[:], AF.Abs, scale=1.0 / HUBER)
            mb = pool.tile([P, 3 * W2], f32)
            nc.gpsimd.tensor_scalar(mb[:], xb[:], 1.0, None, OP.min)
            tb = pool.tile([P, 3 * W2], f32)
            nc.vector.scalar_tensor_tensor(tb[:], mb[:], -0.5, xb[:],
                                           OP.mult, OP.add)
            nc.vector.tensor_tensor(tb[:], tb[:], mb[:], OP.mult)
            part = pool.tile([P, 2], f32)
            tb3 = tb.rearrange("p (c n) -> p c n", c=3)
            nc.vector.tensor_reduce(part[:, 0:1], tb3[:, :, 0:128], AX.XY, OP.add)
            nc.vector.tensor_reduce(part[:, 1:2], tb3[:, :, 128:192], AX.XY, OP.add)
            nc.sync.dma_start(o_d[:], part[:])

            if debug:
                for name, t in [("dbg_S45", S45), ("dbg_C45", C45),
                                ("dbg_gh", gh), ("dbg_dq", dq),
                                ("dbg_rw", rw), ("dbg_rv", rv),
                                ("dbg_rs", rs), ("dbg_S1", S1),
                                ("dbg_C1", C1)]:
                    dd = nc.declare_dram_parameter(name, list(t[:].shape), f32,
                                                   isOutput=True)
                    nc.sync.dma_start(dd[:], t[:])

    nc.compile()
    return nc


def _get_nc():
    if "nc" not in _CACHE:
        _CACHE["nc"] = _build()
    return _CACHE["nc"]


def shard_inputs(w_hat, dw_16):
    """full inputs -> list of per-core {'w','d'} maps."""
    maps = []
    for c in range(NCORES):
        wc = np.ascontiguousarray(
            w_hat[c * SPC:(c + 1) * SPC].reshape(SPC, 16, WCOLS)
        ).reshape(P, WCOLS)
        dc = np.ascontiguousarray(
            dw_16[c * SPC:(c + 1) * SPC, ::16].reshape(SPC, 16, NB16, 3)
        ).reshape(P, DCOLS)
        maps.append({"w": wc, "d": dc})
    return maps


def _huber_sum_f32(rs_flat):
    """Same f32 ops as the device huber; rs_flat: f32 array."""
    x = (np.abs(rs_flat) * np.float32(1.0 / HUBER)).astype(np.float32)
    m = np.minimum(x, np.float32(1.0))
    t = (m * np.float32(-0.5) + x).astype(np.float32)
    return (m * t).astype(np.float32).sum(dtype=np.float64)


def combine_outputs(outs):
    """list of per-core {'out': (128,2), 'skip': (8,30)} -> scalar loss."""
    s16 = 0.0
    s32 = 0.0
    for om in outs:
        o = np.asarray(om["out"], dtype=np.float64)
        s16 += o[:, 0].sum()
        s32 += o[:, 1].sum()
        sk = np.asarray(om["skip"], dtype=np.float32)
        s16 -= _huber_sum_f32(sk[:, 0:3 * N0])
        s32 -= _huber_sum_f32(sk[:, 3 * N0:])
    c16 = NSEQ * (T // 16 - N0) * 3
    c32 = NSEQ * (T // 32 - N0) * 3
    loss = WLOSS * HUBER ** 2 * (s16 / c16) + WLOSS * HUBER ** 2 * (s32 / c32) / 4.0
    return np.float32(loss)


def kernel(w_hat, dw_16):
    from concourse.bass_utils import run_bass_kernel_spmd

    w_hat = np.asarray(w_hat, dtype=np.float32)
    dw_16 = np.asarray(dw_16, dtype=np.float32)
    nc = _get_nc()
    in_maps = shard_inputs(w_hat, dw_16)
    res = run_bass_kernel_spmd(nc, in_maps, list(range(NCORES)))
    return combine_outputs(res.results)


# revision 19
# speedup vs baseline: 1.2371x; 1.2371x over previous
"""Trainium2 Bass kernel for nn_DGLossVer1 (SO(3) gyro loss).

Math: the product of 16 (or 32) small-rotation exponentials exp(dt*w_i) is
composed via the 2nd-order BCH formula Z = dt*S + (dt^2/2)*C with
S = sum(u_i), C = sum_{i<j} u_i x u_j, computed by a pairwise tree
(C_AB = C_A + C_B + S_A x S_B).  The block rotation is kept as an
UNNORMALIZED quaternion (1, tan(|Z|/2)/|Z| * Z); everything downstream
(relative rotation, log) is scale-invariant, so no normalization anywhere.
The log mirrors the reference: ang = arccos(clip(2w^2-1)), coef =
0.5*ang/sin(ang), phi = coef*4*w*v (all in the scale-free form), with
arccos(c) = pi/2 - arctan(c*rsqrt(1-c^2)).  rsqrt = bit-trick seed + 2
Newton iterations (ACT Rsqrt/Reciprocal are banned for accuracy).

Sharding: pure data-parallel, 8 sequences per core; each core returns two
partial Huber sums per partition; host does the tiny weighted reduction.
"""
import numpy as np

P = 128
DT = 0.005
WLOSS = 1.0e6
HUBER = 0.005
N0 = 5
NSEQ, T = 64, 32768
NCORES = 8
SPC = NSEQ // NCORES          # sequences per core
STEPS = SPC * T // P          # 2048 steps per partition
WCOLS = STEPS * 3             # 6144
NB16 = STEPS // 16            # 128 16-blocks per partition
NB32 = STEPS // 32            # 64
DCOLS = NB16 * 3              # 384
N1 = STEPS // 2               # 1024 pairs per partition

_CACHE = {}


def _build(debug=False):
    import concourse.bass as bass
    import concourse.tile as tile
    import concourse.mybir as mybir
    from concourse import bacc

    f32 = mybir.dt.float32
    i32 = mybir.dt.int32
    AF = mybir.ActivationFunctionType
    OP = mybir.AluOpType
    AX = mybir.AxisListType

    nc = bacc.Bacc(None)
    w_d = nc.declare_dram_parameter("w", [P, WCOLS], f32, isOutput=False)
    d_d = nc.declare_dram_parameter("d", [P, DCOLS], f32, isOutput=False)
    o_d = nc.declare_dram_parameter("out", [P, 2], f32, isOutput=True)

    TT, TS, STT = OP.mult, None, None  # readability only

    with tile.TileContext(nc) as tc:
        with tc.tile_pool(name="main", bufs=1) as pool:
            # ---- input DMA (w in 2 chunks for overlap) ----
            w0 = pool.tile([P, WCOLS // 2], f32)
            w1 = pool.tile([P, WCOLS // 2], f32)
            d = pool.tile([P, DCOLS], f32)
            nc.sync.dma_start(w0[:], w_d[:, 0:WCOLS // 2])
            nc.sync.dma_start(d[:], d_d[:])
            nc.sync.dma_start(w1[:], w_d[:, WCOLS // 2:])

            hpi = pool.tile([P, 1], f32)
            nc.gpsimd.memset(hpi[:], float(np.pi / 2))
            fpi = pool.tile([P, 1], f32)
            nc.gpsimd.memset(fpi[:], float(np.pi))

            def rsqrt2(x_ap, n, out_t, scr_t, eng_tt=nc.vector, iters=2):
                """out = rsqrt(x) via bit trick + Newton iters.
                x_ap: (P, n) AP (positive); out_t/scr_t: (P, n) f32 tiles."""
                oi = out_t[:, 0:n].bitcast(i32)
                si = scr_t[:, 0:n].bitcast(i32)
                nc.vector.tensor_scalar(si, x_ap.bitcast(i32), 1, -1,
                                        OP.arith_shift_right, OP.bitwise_xor)
                nc.vector.tensor_scalar(oi, si, 0x5F3759E0, None, OP.add)
                y = out_t[:, 0:n]
                s = scr_t[:, 0:n]
                for _ in range(iters):
                    eng_tt.tensor_tensor(s, y, y, OP.mult)
                    eng_tt.tensor_tensor(s, s, x_ap, OP.mult)
                    nc.scalar.activation(s, s, AF.Copy, bias=1.5, scale=-0.5)
                    eng_tt.tensor_tensor(y, y, s, OP.mult)
                return y

            # ---- k1: pairs of steps; host provides comp/parity-planar chunks:
            # chunk tile cols = c*1024 + parity*512 + pair  (all unit-stride)
            S1 = pool.tile([P, 3 * N1], f32)
            C1 = pool.tile([P, 3 * N1], f32)
            CR = pool.tile([P, 3 * N1], f32)  # cross scratch, reused at all levels
            H = N1 // 2  # pairs per chunk
            for h, wt in ((0, w0), (1, w1)):
                def we(c):  # even-step comp c view, this chunk
                    return wt[:, c * 2 * H:c * 2 * H + H]
                def wo(c):
                    return wt[:, c * 2 * H + H:(c + 1) * 2 * H]
                for c in range(3):
                    eng = nc.gpsimd if c == 0 else nc.vector
                    a, b = (c + 1) % 3, (c + 2) % 3
                    dst = C1[:, c * N1 + h * H:c * N1 + (h + 1) * H]
                    scr = CR[:, c * N1 + h * H:c * N1 + (h + 1) * H]
                    eng.tensor_tensor(dst, we(a), wo(b), OP.mult)
                    eng.tensor_tensor(scr, we(b), wo(a), OP.mult)
                    eng.tensor_tensor(dst, dst, scr, OP.subtract)
                # S1 3-comp merged add: even-plane + odd-plane (unit stride)
                s_out = S1.rearrange("p (c n) -> p c n", c=3)[:, :, h * H:(h + 1) * H]
                w3 = wt.rearrange("p (c q n) -> p c q n", c=3, q=2)
                nc.vector.tensor_tensor(
                    s_out, w3[:, :, 0, :], w3[:, :, 1, :], OP.add)

            # ---- k2..k5 ----
            def level(Sp, Cp, n_in, Sn, Cn, s_off=0, d_off=0):
                """pairwise combine: n_in -> n_in//2 per comp.
                Sp/Cp views are comp-blocked tiles with block stride bs_in,
                writing into Sn/Cn at comp block stride bs_out + d_off."""
                n = n_in // 2
                for c in range(3):
                    eng = nc.gpsimd if c == 0 else nc.vector
                    a, b = (c + 1) % 3, (c + 2) % 3
                    def ev(t, cc, bs):
                        return t.rearrange("p (c n) -> p c n", c=3)[:, cc, s_off:s_off + n_in:2]
                    def od(t, cc, bs):
                        return t.rearrange("p (c n) -> p c n", c=3)[:, cc, s_off + 1:s_off + n_in:2]
                    dst = Cn.rearrange("p (c n) -> p c n", c=3)[:, c, d_off:d_off + n]
                    scr = CR[:, c * N1:c * N1 + n]
                    eng.tensor_tensor(dst, ev(Sp, a, 0), od(Sp, b, 0), OP.mult)
                    eng.tensor_tensor(scr, ev(Sp, b, 0), od(Sp, a, 0), OP.mult)
                    eng.tensor_tensor(dst, dst, scr, OP.subtract)
                # C += Ce + Co (3-comp merged)
                c3 = Cn.rearrange("p (c n) -> p c n", c=3)[:, :, d_off:d_off + n]
                cp3 = Cp.rearrange("p (c n) -> p c n", c=3)
                s3 = Sn.rearrange("p (c n) -> p c n", c=3)[:, :, d_off:d_off + n]
                sp3 = Sp.rearrange("p (c n) -> p c n", c=3)
                ce = cp3[:, :, s_off:s_off + n_in:2]
                co = cp3[:, :, s_off + 1:s_off + n_in:2]
                sc = CR.rearrange("p (c n) -> p c n", c=3)[:, :, 0:n]
                nc.vector.tensor_tensor(sc, ce, co, OP.add)
                nc.vector.tensor_tensor(c3, c3, sc, OP.add)
                nc.vector.tensor_tensor(
                    s3, sp3[:, :, s_off:s_off + n_in:2],
                    sp3[:, :, s_off + 1:s_off + n_in:2], OP.add)

            S2 = pool.tile([P, 3 * 512], f32)
            C2 = pool.tile([P, 3 * 512], f32)
            S3 = pool.tile([P, 3 * 256], f32)
            C3 = pool.tile([P, 3 * 256], f32)
            # S45/C45: (P, 3*192): per comp [16-level 0:128 | 32-level 128:192]
            S45 = pool.tile([P, 3 * 192], f32)
            C45 = pool.tile([P, 3 * 192], f32)
            level(S1, C1, N1, S2, C2)
            level(S2, C2, 512, S3, C3)
            level(S3, C3, 256, S45, C45)          # k4 -> NB16 at d_off 0
            level(S45, C45, 128, S45, C45, d_off=128)  # k5 reads [0:128) writes [128:192)

            # ---- Z, tan-poly, gh (homogeneous quat vector part) ----
            Z = pool.tile([P, 3 * 192], f32)
            nc.vector.scalar_tensor_tensor(Z[:], C45[:], DT / 2, S45[:],
                                           OP.mult, OP.add)
            sqz = pool.tile([P, 3 * 192], f32)
            nc.scalar.activation(sqz[:], Z[:], AF.Square)
            n2z = pool.tile([P, 192], f32)
            nc.vector.tensor_tensor(n2z[:], sqz[:, 0:192], sqz[:, 192:384], OP.add)
            nc.vector.tensor_tensor(n2z[:], n2z[:], sqz[:, 384:576], OP.add)
            tp = pool.tile([P, 192], f32)
            nc.scalar.activation(tp[:], n2z[:], AF.Copy, bias=DT ** 2 / 24,
                                 scale=DT ** 4 / 240)
            nc.vector.tensor_tensor(tp[:], tp[:], n2z[:], OP.mult)
            nc.scalar.activation(tp[:], tp[:], AF.Copy, bias=0.5 * DT, scale=DT)
            gh = pool.tile([P, 3 * 192], f32)
            tpb = tp[:].unsqueeze(1).broadcast_to([P, 3, 192])
            nc.vector.tensor_tensor(gh.rearrange("p (c n) -> p c n", c=3),
                                    tpb, Z.rearrange("p (c n) -> p c n", c=3),
                                    OP.mult)

            # ---- d16 exp (true unit quats via Sin table) ----
            # dq: (P, 4*192): [w | x | y | z], each [d16 0:128 | d32 128:192]
            dq = pool.tile([P, 4 * 192], f32)
            sqd = pool.tile([P, DCOLS], f32)
            nc.scalar.activation(sqd[:], d[:], AF.Square)
            n2d = pool.tile([P, NB16], f32)
            nc.vector.tensor_tensor(n2d[:], sqd[:, 0:DCOLS:3], sqd[:, 1:DCOLS:3], OP.add)
            nc.vector.tensor_tensor(n2d[:], n2d[:], sqd[:, 2:DCOLS:3], OP.add)
            nc.vector.tensor_scalar(n2d[:], n2d[:], 1e-30, None, OP.max)
            y1t = pool.tile([P, NB16], f32)
            scr1 = pool.tile([P, NB16], f32)
            y1 = rsqrt2(n2d[:], NB16, y1t, scr1, iters=3)
            th = pool.tile([P, NB16], f32)
            nc.vector.tensor_tensor(th[:], n2d[:], y1, OP.mult)
            # cos(th/2) = sin(pi/2 - th/2); sin(th/2) = sin(pi - th/2)
            # (keeps the argument inside the Sin table's [-pi, pi] domain)
            nc.scalar.activation(dq[:, 0:128], th[:], AF.Sin, bias=hpi[:], scale=-0.5)
            s0 = pool.tile([P, NB16], f32)
            nc.scalar.activation(s0[:], th[:], AF.Sin, bias=fpi[:], scale=-0.5)
            nc.vector.tensor_tensor(s0[:], s0[:], y1, OP.mult)
            s0b = s0[:].unsqueeze(1).broadcast_to([P, 3, NB16])
            dqv16 = dq.rearrange("p (c n) -> p c n", c=4)[:, 1:4, 0:128]
            d3 = d.rearrange("p (j c) -> p c j", c=3)
            nc.vector.tensor_tensor(dqv16, s0b, d3, OP.mult)

            # ---- d32 = qmul(d16 even, d16 odd) into dq[:, *, 128:192] ----
            dq4 = dq.rearrange("p (c n) -> p c n", c=4)
            q1 = dq4[:, :, 0:128:2]
            q2 = dq4[:, :, 1:128:2]
            pp = pool.tile([P, 4 * NB32], f32)
            pp4 = pp.rearrange("p (c n) -> p c n", c=4)
            nc.vector.tensor_tensor(pp4, q1, q2, OP.mult)
            w32 = dq[:, 128:192]
            nc.vector.tensor_tensor(w32, pp[:, 0:64], pp[:, 64:128], OP.subtract)
            nc.vector.tensor_tensor(w32, w32, pp[:, 128:192], OP.subtract)
            nc.vector.tensor_tensor(w32, w32, pp[:, 192:256], OP.subtract)
            w1b = dq[:, 0:128:2].unsqueeze(1).broadcast_to([P, 3, NB32])
            w2b = dq[:, 1:128:2].unsqueeze(1).broadcast_to([P, 3, NB32])
            v1 = dq4[:, 1:4, 0:128:2]
            v2 = dq4[:, 1:4, 1:128:2]
            t1 = pool.tile([P, 3 * NB32], f32)
            t13 = t1.rearrange("p (c n) -> p c n", c=3)
            t2 = pool.tile([P, 3 * NB32], f32)
            t23 = t2.rearrange("p (c n) -> p c n", c=3)
            nc.vector.tensor_tensor(t13, w1b, v2, OP.mult)
            nc.vector.tensor_tensor(t23, w2b, v1, OP.mult)
            nc.vector.tensor_tensor(t1[:], t1[:], t2[:], OP.add)
            cr32 = pool.tile([P, 3 * NB32], f32)
            for c in range(3):
                a, b = (c + 1) % 3, (c + 2) % 3
                dst = cr32[:, c * NB32:(c + 1) * NB32]
                scr = t2[:, c * NB32:(c + 1) * NB32]
                nc.gpsimd.tensor_tensor(dst, v1[:, a], v2[:, b], OP.mult)
                nc.gpsimd.tensor_tensor(scr, v1[:, b], v2[:, a], OP.mult)
                nc.gpsimd.tensor_tensor(dst, dst, scr, OP.subtract)
            nc.vector.tensor_tensor(t1[:], t1[:], cr32[:], OP.add)
            dqv32 = dq4[:, 1:4, 128:192]
            nc.vector.tensor_copy(dqv32, t13)

            # ---- rel = conj(1, gh) x dq   (width 192) ----
            gh3 = gh.rearrange("p (c n) -> p c n", c=3)
            dqv = dq4[:, 1:4, :]
            dm = pool.tile([P, 3 * 192], f32)
            nc.vector.tensor_tensor(dm.rearrange("p (c n) -> p c n", c=3),
                                    gh3, dqv, OP.mult)
            rw = pool.tile([P, 192], f32)
            nc.vector.tensor_tensor(rw[:], dm[:, 0:192], dm[:, 192:384], OP.add)
            nc.vector.tensor_tensor(rw[:], rw[:], dm[:, 384:576], OP.add)
            nc.vector.tensor_tensor(rw[:], rw[:], dq[:, 0:192], OP.add)
            cwb = dq[:, 0:192].unsqueeze(1).broadcast_to([P, 3, 192])
            rv = pool.tile([P, 3 * 192], f32)
            rv3 = rv.rearrange("p (c n) -> p c n", c=3)
            nc.vector.tensor_tensor(rv3, cwb, gh3, OP.mult)
            nc.vector.tensor_tensor(rv[:], dq[:, 192:], rv[:], OP.subtract)
            crr = pool.tile([P, 3 * 192], f32)
            for c in range(3):
                a, b = (c + 1) % 3, (c + 2) % 3
                dst = crr[:, c * 192:(c + 1) * 192]
                scr = dm[:, c * 192:(c + 1) * 192]
                nc.gpsimd.tensor_tensor(dst, gh3[:, a], dqv[:, b], OP.mult)
                nc.gpsimd.tensor_tensor(scr, gh3[:, b], dqv[:, a], OP.mult)
                nc.gpsimd.tensor_tensor(dst, dst, scr, OP.subtract)
            nc.vector.tensor_tensor(rv[:], rv[:], crr[:], OP.subtract)

            # ---- log (mirrors reference clip semantics, scale-free) ----
            W2 = 192
            sqv = pool.tile([P, 3 * W2], f32)
            nc.scalar.activation(sqv[:], rv[:], AF.Square)
            n2v = pool.tile([P, W2], f32)
            nc.vector.tensor_tensor(n2v[:], sqv[:, 0:192], sqv[:, 192:384], OP.add)
            nc.vector.tensor_tensor(n2v[:], n2v[:], sqv[:, 384:576], OP.add)
            w2t = pool.tile([P, W2], f32)
            nc.scalar.activation(w2t[:], rw[:], AF.Square)
            q2t = pool.tile([P, W2], f32)
            nc.vector.tensor_tensor(q2t[:], w2t[:], n2v[:], OP.add)
            rt = pool.tile([P, W2], f32)
            rscr = pool.tile([P, W2], f32)
            r = rsqrt2(q2t[:], W2, rt, rscr)
            rq = pool.tile([P, W2], f32)
            nc.vector.tensor_tensor(rq[:], r, r, OP.mult)      # ~1/q2
            # one reciprocal-Newton to bring rq to ~1-ULP: rq *= (2 - q2*rq)
            nc.vector.tensor_tensor(rscr[:], q2t[:], rq[:], OP.mult)
            nc.scalar.activation(rscr[:], rscr[:], AF.Copy, bias=2.0, scale=-1.0)
            nc.vector.tensor_tensor(rq[:], rq[:], rscr[:], OP.mult)
            cost = pool.tile([P, W2], f32)
            nc.vector.tensor_tensor(cost[:], w2t[:], n2v[:], OP.subtract)
            nc.vector.tensor_tensor(cost[:], cost[:], rq[:], OP.mult)
            CLIP = 1.0 - 1e-7
            nc.vector.tensor_scalar(cost[:], cost[:], CLIP, -CLIP, OP.min, OP.max)
            c2t = pool.tile([P, W2], f32)
            nc.scalar.activation(c2t[:], cost[:], AF.Square)
            nc.scalar.activation(c2t[:], c2t[:], AF.Copy, bias=1.0, scale=-1.0)
            rs2t = pool.tile([P, W2], f32)
            rs2 = rsqrt2(c2t[:], W2, rs2t, rscr)               # 1/sin(ang)
            # F = 0.5*arccos(cos)/sin(arccos(cos)) via deg-10 poly in t=|cos|-1
            # (F is analytic at cos=1; the cos<0 half uses F(-c) and pi/2*rs2)
            KP = [0.5000000010056445, -0.1666664296147386, 0.06667585538901223,
                  -0.028433366986487976, 0.013753622162797092,
                  -0.0011196834360748097, 0.015245614903288171,
                  0.020070084287574758, 0.02282400093211004,
                  0.01299667485963209, 0.0037463467111214254]
            tpoly = pool.tile([P, W2], f32)
            nc.scalar.activation(tpoly[:], cost[:], AF.Abs)
            nc.scalar.activation(tpoly[:], tpoly[:], AF.Copy, bias=-1.0)  # t = |c|-1
            t2p = pool.tile([P, W2], f32)
            t4p = pool.tile([P, W2], f32)
            nc.vector.tensor_tensor(t2p[:], tpoly[:], tpoly[:], OP.mult)
            nc.vector.tensor_tensor(t4p[:], t2p[:], t2p[:], OP.mult)
            e0 = pool.tile([P, W2], f32)
            e1 = pool.tile([P, W2], f32)
            e2 = pool.tile([P, W2], f32)
            e3 = pool.tile([P, W2], f32)
            e4 = pool.tile([P, W2], f32)
            nc.scalar.activation(e0[:], tpoly[:], AF.Copy, bias=KP[0], scale=KP[1])
            nc.scalar.activation(e1[:], tpoly[:], AF.Copy, bias=KP[2], scale=KP[3])
            nc.scalar.activation(e2[:], tpoly[:], AF.Copy, bias=KP[4], scale=KP[5])
            nc.scalar.activation(e3[:], tpoly[:], AF.Copy, bias=KP[6], scale=KP[7])
            nc.scalar.activation(e4[:], tpoly[:], AF.Copy, bias=KP[8], scale=KP[9])
            g2 = pool.tile([P, W2], f32)
            nc.scalar.activation(g2[:], t2p[:], AF.Copy, scale=KP[10])
            nc.vector.tensor_tensor(g2[:], g2[:], e4[:], OP.add)     # f2
            nc.vector.tensor_tensor(e1[:], e1[:], t2p[:], OP.mult)
            nc.vector.tensor_tensor(e0[:], e0[:], e1[:], OP.add)     # f0
            nc.vector.tensor_tensor(e3[:], e3[:], t2p[:], OP.mult)
            nc.vector.tensor_tensor(e2[:], e2[:], e3[:], OP.add)     # f1
            nc.vector.tensor_tensor(g2[:], g2[:], t4p[:], OP.mult)
            nc.vector.tensor_tensor(g2[:], g2[:], e2[:], OP.add)
            nc.vector.tensor_tensor(g2[:], g2[:], t4p[:], OP.mult)
            nc.vector.tensor_tensor(g2[:], g2[:], e0[:], OP.add)     # p = F(|c|)
            sgn = pool.tile([P, W2], f32)
            nc.scalar.activation(sgn[:], cost[:], AF.Sign)
            u1 = pool.tile([P, W2], f32)
            nc.scalar.activation(u1[:], sgn[:], AF.Copy, bias=float(np.pi / 4),
                                 scale=float(-np.pi / 4))
            nc.vector.tensor_tensor(u1[:], u1[:], rs2, OP.mult)
            nc.vector.tensor_tensor(g2[:], g2[:], sgn[:], OP.mult)
            cf = pool.tile([P, W2], f32)
            nc.vector.tensor_tensor(cf[:], u1[:], g2[:], OP.add)     # 0.5*ang/sin
            cf2 = pool.tile([P, W2], f32)
            nc.vector.scalar_tensor_tensor(cf2[:], rw[:], 4.0, rq[:], OP.mult, OP.mult)
            nc.vector.tensor_tensor(cf[:], cf[:], cf2[:], OP.mult)
            rs = pool.tile([P, 3 * W2], f32)
            cfb = cf[:].unsqueeze(1).broadcast_to([P, 3, W2])
            nc.vector.tensor_tensor(rs.rearrange("p (c n) -> p c n", c=3),
                                    cfb, rv3, OP.mult)

            # ---- huber + partial sums ----
            # The N0-skip is handled host-side: ship the skipped rs values out
            # and subtract their huber contribution from the partial sums.
            rs3 = rs.rearrange("p (c n) -> p c n", c=3)
            skip_d = nc.declare_dram_parameter("skip", [SPC, 2 * 3 * N0], f32,
                                               isOutput=True)
            nc.sync.dma_start(skip_d[:, 0:3 * N0], rs3[0:P:16, :, 0:N0])
            nc.sync.dma_start(skip_d[:, 3 * N0:], rs3[0:P:16, :, 128:128 + N0])
            xb = pool.tile([P, 3 * W2], f32)
            nc.scalar.activation(xb[:], rs[:], AF.Abs, scale=1.0 / HUBER)
            mb = pool.tile([P, 3 * W2], f32)
            nc.vector.tensor_scalar(mb[:], xb[:], 1.0, None, OP.min)
            tb = pool.tile([P, 3 * W2], f32)
            nc.vector.scalar_tensor_tensor(tb[:], mb[:], -0.5, xb[:],
                                           OP.mult, OP.add)
            nc.vector.tensor_tensor(tb[:], tb[:], mb[:], OP.mult)
            part = pool.tile([P, 2], f32)
            tb3 = tb.rearrange("p (c n) -> p c n", c=3)
            nc.vector.tensor_reduce(part[:, 0:1], tb3[:, :, 0:128], AX.XY, OP.add)
            nc.vector.tensor_reduce(part[:, 1:2], tb3[:, :, 128:192], AX.XY, OP.add)
            nc.sync.dma_start(o_d[:], part[:])

            if debug:
                for name, t in [("dbg_S45", S45), ("dbg_C45", C45),
                                ("dbg_gh", gh), ("dbg_dq", dq),
                                ("dbg_rw", rw), ("dbg_rv", rv),
                                ("dbg_rs", rs), ("dbg_S1", S1),
                                ("dbg_C1", C1)]:
                    dd = nc.declare_dram_parameter(name, list(t[:].shape), f32,
                                                   isOutput=True)
                    nc.sync.dma_start(dd[:], t[:])

    nc.compile()
    return nc


def _get_nc():
    if "nc" not in _CACHE:
        _CACHE["nc"] = _build()
    return _CACHE["nc"]


def shard_inputs(w_hat, dw_16):
    """full inputs -> list of per-core {'w','d'} maps."""
    maps = []
    H = N1 // 2
    for c in range(NCORES):
        # (P, chunk=2, pair=H, parity=2, comp=3) -> (P, chunk, comp, parity, pair)
        wc = w_hat[c * SPC:(c + 1) * SPC].reshape(P, 2, H, 2, 3)
        wc = np.ascontiguousarray(wc.transpose(0, 1, 4, 3, 2)).reshape(P, WCOLS)
        dc = np.ascontiguousarray(
            dw_16[c * SPC:(c + 1) * SPC, ::16].reshape(SPC, 16, NB16, 3)
        ).reshape(P, DCOLS)
        maps.append({"w": wc, "d": dc})
    return maps


def _huber_sum_f32(rs_flat):
    """Same f32 ops as the device huber; rs_flat: f32 array."""
    x = (np.abs(rs_flat) * np.float32(1.0 / HUBER)).astype(np.float32)
    m = np.minimum(x, np.float32(1.0))
    t = (m * np.float32(-0.5) + x).astype(np.float32)
    return (m * t).astype(np.float32).sum(dtype=np.float64)


def combine_outputs(outs):
    """list of per-core {'out': (128,2), 'skip': (8,30)} -> scalar loss."""
    s16 = 0.0
    s32 = 0.0
    for om in outs:
        o = np.asarray(om["out"], dtype=np.float64)
        s16 += o[:, 0].sum()
        s32 += o[:, 1].sum()
        sk = np.asarray(om["skip"], dtype=np.float32)
        s16 -= _huber_sum_f32(sk[:, 0:3 * N0])
        s32 -= _huber_sum_f32(sk[:, 3 * N0:])
    c16 = NSEQ * (T // 16 - N0) * 3
    c32 = NSEQ * (T // 32 - N0) * 3
    loss = WLOSS * HUBER ** 2 * (s16 / c16) + WLOSS * HUBER ** 2 * (s32 / c32) / 4.0
    return np.float32(loss)


def kernel(w_hat, dw_16):
    from concourse.bass_utils import run_bass_kernel_spmd

    w_hat = np.asarray(w_hat, dtype=np.float32)
    dw_16 = np.asarray(dw_16, dtype=np.float32)
    nc = _get_nc()
    in_maps = shard_inputs(w_hat, dw_16)
    res = run_bass_kernel_spmd(nc, in_maps, list(range(NCORES)))
    return combine_outputs(res.results)
